# revision 8
# baseline (speedup 1.0000x reference)
"""Causal self-attention with rotary embeddings on 8 Trainium2 NeuronCores.

Hybrid batch+head tensor parallel: core m handles batch m//4 and heads
[4*(m%4), 4*(m%4)+4).  Each core reads only its batch's x (4 MB), computes
qkv for its 4 heads, rotary, causal attention, and a partial output
projection with its 256 rows of w_proj; the host sums the 4 partial
outputs per batch.

Per-core device layout (heads grouped in 2 pairs):
  - Q^T/K^T per pair: [128 rows = head_a(64) | head_b(64), t].  Within a
    head the 64 dims are permuted to [evens(32), odds(32)] (host permutes
    the w_attn columns), making rotary 3 fp16 DVE tensor ops plus a
    32-row block swap done by SBUF->SBUF DMA.  Scores are invariant to a
    shared d-permutation of Q and K.
  - Attention is chunk-major (512 queries at a time) per head so scores
    (PE) / exp (Act) / P@V (PE) of many (head, chunk, block) units stay
    in flight together.  Scores stream only the causal range; P@V uses
    exact diagonal sub-ranges.  A ones-augmented V makes row 64 of the
    P@V accumulation the softmax denominator.  No max-subtraction
    (scores are O(6); fp32 exp is safe).
  - V is computed group-major like Q/K (full-width streams, weight loads
    hidden) and transposed to t-major on the PE (cheap 128-col passes).
  - The output projection for a chunk runs right after the last head's
    normalize, so the y DMA streams through phase 2 instead of forming
    a tail.

All matmul inputs fp16 (1 cyc/row on PE); accumulation fp32 in PSUM.
"""

import numpy as np

B, T, C, H = 2, 2048, 1024, 16
HD = C // H            # 64
N_CORES = 8
CPB = 4                # cores per batch
HPC = 4                # heads per core (2 pairs)
TC = 512               # t-chunk for phase 1
NTC = T // TC          # 4
KB = 128               # k-block
NKB = T // KB          # 16
QC = 512               # q-chunk for attention/projection
NQC = T // QC          # 4

_CACHE = {}


def _build_bass():
    import concourse.bacc as bacc
    import concourse.mybir as mybir
    import concourse.tile as tile
    from concourse.masks import make_identity, make_upper_triangular

    f16 = mybir.dt.float16
    f32 = mybir.dt.float32
    Exp = mybir.ActivationFunctionType.Exp
    Copy = mybir.ActivationFunctionType.Copy
    mult = mybir.AluOpType.mult

    nc = bacc.Bacc()

    xT = nc.dram_tensor("xT", [C, T], f16, kind="ExternalInput")
    wqkv = nc.dram_tensor("wqkv", [C, 768], f16, kind="ExternalInput")
    wp = nc.dram_tensor("wp", [2 * 128, C], f16, kind="ExternalInput")
    trig1 = nc.dram_tensor("trig1", [128, T], f16, kind="ExternalInput")
    trig2 = nc.dram_tensor("trig2", [128, T], f16, kind="ExternalInput")
    y = nc.dram_tensor("y", [T, C], f16, kind="ExternalOutput")

    CCH = C // 128  # 8 contraction chunks

    with tile.TileContext(nc) as tc:
        with (
            tc.tile_pool(name="const", bufs=1) as const,
            tc.tile_pool(name="persist", bufs=1) as persist,
            tc.tile_pool(name="stream", bufs=2) as stream,
            tc.tile_pool(name="ptp", bufs=8) as ptp,
            tc.tile_pool(name="psum", bufs=1, space="PSUM") as psum,
        ):
            # ---- constants; x chunk 0 interleaved with wqkv so the first
            # matmul starts early ----
            wqkv_sb = const.tile([128, CCH, 768], f16)
            wqkv_r = wqkv.rearrange("(cc p) j -> p cc j", p=128)
            x_sb0 = stream.tile([128, CCH, TC], f16, tag="x", name="x_sb")
            xT_r = xT.rearrange("(cc p) t -> p cc t", p=128)
            for cc in range(CCH):
                nc.sync.dma_start(out=wqkv_sb[:, cc, :], in_=wqkv_r[:, cc, :])
                nc.sync.dma_start(out=x_sb0[:, cc, :], in_=xT_r[:, cc, 0:TC])
            trig1_sb = const.tile([128, T], f16)
            nc.scalar.dma_start(out=trig1_sb, in_=trig1[:, :])
            trig2_sb = const.tile([128, T], f16)
            nc.scalar.dma_start(out=trig2_sb, in_=trig2[:, :])
            wp_sb = const.tile([128, 2, C], f16)
            wp_r = wp.rearrange("(p2 p) c -> p p2 c", p=128)
            nc.sync.dma_start(out=wp_sb, in_=wp_r)
            # mask[k, q] = 1 where q >= k (keep), 0 where q < k
            mask_ut = const.tile([128, 128], f16)
            make_upper_triangular(nc, mask_ut, val=1.0, diag=True)
            ident = const.tile([128, 128], f16)
            make_identity(nc, ident)

            # ---- persistent tensors ----
            QrotT = persist.tile([128, 2, T], f16)
            KrotT = persist.tile([128, 2, T], f16)
            # V in t-major per (pair, k-block): [V_a(64) | ones | V_b(64) | ones]
            Vaug = persist.tile([128, 2, NKB, 130], f16)
            ones_cols = Vaug.rearrange(
                "pp q J (h x) -> pp q J h x", x=65)[:, :, :, :, 64]
            nc.gpsimd.memset(ones_cols, 1.0)
            Yn = persist.tile([128, 2, T], f16)

            # ================= phase 1: qkv + rotary ======================
            for i in range(NTC):
                ts = slice(i * TC, (i + 1) * TC)
                if i == 0:
                    x_sb = x_sb0
                else:
                    x_sb = stream.tile([128, CCH, TC], f16, tag="x",
                                       name="x_sb")
                    for cc in range(CCH):
                        nc.sync.dma_start(out=x_sb[:, cc, :],
                                          in_=xT_r[:, cc, ts])

                for g in range(4):      # Qp0 Qp1 Kp0 Kp1
                    dst = QrotT if g < 2 else KrotT
                    p = g % 2
                    acc = psum.tile([128, TC], f32, tag="acc", bufs=2,
                                    name="acc")
                    for cc in range(CCH):
                        nc.tensor.matmul(
                            acc, wqkv_sb[:, cc, g * 128:(g + 1) * 128],
                            x_sb[:, cc, :],
                            start=(cc == 0), stop=(cc == CCH - 1))
                    g16 = stream.tile([128, TC], f16, tag="g16")
                    nc.vector.tensor_copy(g16, acc)
                    # 32-row block swap (evens <-> odds per head) via DMA
                    gsw = stream.tile([128, TC], f16, tag="gsw")
                    for blk in range(4):
                        src = blk ^ 1
                        nc.sync.dma_start(
                            out=gsw[blk * 32:(blk + 1) * 32, :],
                            in_=g16[src * 32:(src + 1) * 32, :])
                    m1 = stream.tile([128, TC], f16, tag="m1")
                    nc.vector.tensor_mul(m1, g16, trig1_sb[:, ts])
                    m2 = stream.tile([128, TC], f16, tag="m2")
                    nc.vector.tensor_mul(m2, gsw, trig2_sb[:, ts])
                    nc.vector.tensor_add(dst[:, p, ts], m1, m2)

                # V group-major + PE transpose to t-major
                for gv in range(2):     # V pair 0, V pair 1
                    vacc = psum.tile([128, TC], f32, tag="acc", bufs=2,
                                     name="vacc")
                    for cc in range(CCH):
                        nc.tensor.matmul(
                            vacc,
                            wqkv_sb[:, cc, 512 + gv * 128:640 + gv * 128],
                            x_sb[:, cc, :],
                            start=(cc == 0), stop=(cc == CCH - 1))
                    vtmp = stream.tile([128, TC], f16, tag="vtmp")
                    nc.vector.tensor_copy(vtmp, vacc)
                    for tb in range(TC // 128):
                        J = i * 4 + tb
                        vt = psum.tile([128, 128], f16, tag="yps", bufs=2,
                                       name="vt")
                        nc.tensor.transpose(
                            vt, vtmp[:, tb * 128:(tb + 1) * 128], ident)
                        vdst = Vaug.rearrange(
                            "pp q J (h x) -> pp q J h x",
                            x=65)[:, gv, J, :, 0:64]
                        nc.scalar.activation(
                            vdst, vt.rearrange("pp (h x) -> pp h x", h=2),
                            Copy)

            # ================= phase 2: attention, chunk-major ============
            for c in range(NQC):
                cs = slice(c * QC, (c + 1) * QC)
                for u in range(HPC):
                    p, hh = divmod(u, 2)
                    hs = slice(hh * 64, hh * 64 + 64)

                    # scores + exp (+ diag mask) for the chunk's k-blocks
                    pts = []
                    for j in range(4 * c + 4):
                        prefix = max(0, (j - 4 * c) * KB)
                        st = psum.tile([128, QC], f32, tag="st", bufs=2,
                                       name="st")
                        nc.tensor.matmul(
                            st[:, prefix:],
                            KrotT[hs, p, j * KB:(j + 1) * KB],
                            QrotT[hs, p, c * QC + prefix:(c + 1) * QC],
                            start=True, stop=True)
                        pt = ptp.tile([128, QC], f16, tag="pt", name="pt")
                        if prefix:
                            nc.gpsimd.memset(pt[:, 0:prefix], 0.0)
                        nc.scalar.activation(pt[:, prefix:], st[:, prefix:],
                                             Exp)
                        if j >= 4 * c:
                            nc.gpsimd.tensor_mul(
                                pt[:, prefix:prefix + 128],
                                pt[:, prefix:prefix + 128], mask_ut)
                        pts.append(pt)

                    # P@V: start=True on j=0 (full width), stop=True on a
                    # full-width piece, partial diagonal pieces in between
                    if c == 0:
                        order = [(j, j == 0, j == 3, 0) for j in range(4)]
                    else:
                        order = [(j, j == 0, False, 0) for j in range(4 * c)]
                        order += [(j, False, False, (j - 4 * c) * KB)
                                  for j in range(4 * c + 1, 4 * c + 4)]
                        order += [(4 * c, False, True, 0)]
                    yps = psum.tile([128, QC], f32, tag="yps", bufs=2,
                                    name="yps")
                    for j, sa, so, pvlo in order:
                        nc.tensor.matmul(
                            yps[0:65, pvlo:],
                            Vaug[:, p, j, hh * 65:(hh + 1) * 65],
                            pts[j][:, pvlo:],
                            start=sa, stop=so)

                    # normalize rows 0-63 by the ones-row (64)
                    dsb = stream.tile([128, QC], f32, tag="dsb")
                    nc.vector.tensor_copy(dsb[0:1, :], yps[64:65, :])
                    rcp = stream.tile([128, QC], f32, tag="rcp")
                    nc.vector.reciprocal_approx_fast(out=rcp[0:1, :],
                                                     in_=dsb[0:1, :])
                    bc = stream.tile([128, QC], f32, tag="bc")
                    nc.gpsimd.partition_broadcast(bc[0:64, :], rcp[0:1, :])
                    nc.vector.tensor_tensor(
                        out=Yn[hs, p, cs], in0=yps[0:64, :], in1=bc[0:64, :],
                        op=mult)

                # ---- projection + output DMA for this chunk ----
                for tt in range(4 * c, 4 * c + 4):
                    tsl = slice(tt * 128, (tt + 1) * 128)
                    for half in range(2):
                        hsl = slice(half * 512, (half + 1) * 512)
                        pout = psum.tile([128, 512], f32, tag="acc",
                                         bufs=2, name="pout")
                        nc.tensor.matmul(pout, Yn[:, 0, tsl],
                                         wp_sb[:, 0, hsl],
                                         start=True, stop=False)
                        nc.tensor.matmul(pout, Yn[:, 1, tsl],
                                         wp_sb[:, 1, hsl],
                                         start=False, stop=True)
                        yo = stream.tile([128, 512], f16, tag="yo", bufs=4)
                        if half == 0:
                            nc.vector.tensor_copy(yo, pout)
                        else:
                            nc.scalar.activation(yo, pout, Copy)
                        nc.sync.dma_start(out=y[tsl, hsl], in_=yo)

    nc.finalize()
    return nc


def _host_prep(x, cos, sin, w_attn, b_attn, w_proj):
    """Per-core input maps (all fp16)."""
    x = np.asarray(x, dtype=np.float32)
    xT16 = [np.ascontiguousarray(x[b].T).astype(np.float16) for b in range(B)]

    cos = np.asarray(cos, dtype=np.float32)  # [T, 32]
    sin = np.asarray(sin, dtype=np.float32)
    cosF = cos.T.astype(np.float16)          # [32, T]
    sinF = sin.T.astype(np.float16)
    trig1 = np.concatenate([cosF, cosF, cosF, cosF], axis=0)   # [128, T]
    trig2 = np.concatenate([-sinF, sinF, -sinF, sinF], axis=0)

    w_attn = np.asarray(w_attn, dtype=np.float32)
    w_proj = np.asarray(w_proj, dtype=np.float32)
    scale = 1.0 / np.sqrt(HD)

    # per-head column permutation: [even dims, odd dims]
    perm = np.concatenate([np.arange(0, HD, 2), np.arange(1, HD, 2)])

    in_maps = []
    for m in range(N_CORES):
        hb = (m % CPB) * HPC
        cols = []
        for g in range(2):           # Q, K: permuted dims, Q scaled
            for pp in range(2):
                for hh in range(2):
                    hglob = hb + pp * 2 + hh
                    blk = w_attn[:, g * C + hglob * HD:
                                 g * C + (hglob + 1) * HD][:, perm]
                    if g == 0:
                        blk = blk * scale
                    cols.append(blk)
        for hh in range(HPC):        # V: natural dims
            hglob = hb + hh
            cols.append(w_attn[:, 2 * C + hglob * HD:
                               2 * C + (hglob + 1) * HD])
        w_stack = np.concatenate(cols, axis=1).astype(np.float16)
        wp_m = w_proj[hb * HD:(hb + HPC) * HD, :].astype(np.float16)
        in_maps.append({"xT": xT16[m // CPB], "wqkv": w_stack, "wp": wp_m,
                        "trig1": trig1, "trig2": trig2})
    return in_maps


def kernel(x, cos, sin, w_attn, b_attn, w_proj, b_proj):
    from concourse.bass_utils import run_bass_kernel_spmd

    b_attn = np.asarray(b_attn, dtype=np.float32)
    assert not np.any(b_attn), "nonzero b_attn not supported by this kernel"

    in_maps = _host_prep(x, cos, sin, w_attn, b_attn, w_proj)

    if "nc" not in _CACHE:
        _CACHE["nc"] = _build_bass()
    nc = _CACHE["nc"]

    res = run_bass_kernel_spmd(nc, in_maps, core_ids=list(range(N_CORES)))
    _CACHE["last_result"] = res

    y = np.zeros((B, T, C), dtype=np.float64)
    for m in range(N_CORES):
        y[m // CPB] += res.results[m]["y"].astype(np.float64)
    y += np.asarray(b_proj, dtype=np.float64)[None, None, :]
    return y.astype(np.float32)


# revision 9
# speedup vs baseline: 1.6027x; 1.6027x over previous
"""Causal self-attention with rotary embeddings on 8 Trainium2 NeuronCores.

Hybrid batch+head tensor parallel: core m handles batch m//4 and heads
[4*(m%4), 4*(m%4)+4).  Each core reads only its batch's x (4 MB), computes
qkv for its 4 heads, rotary, causal attention, and a partial output
projection with its 256 rows of w_proj; the host sums the 4 partial
outputs per batch.

Per-core device layout (heads grouped in 2 pairs):
  - Q^T/K^T per pair: [128 rows = head_a(64) | head_b(64), t].  Within a
    head the 64 dims are permuted to [evens(32), odds(32)] (host permutes
    the w_attn columns), making rotary 3 fp16 DVE tensor ops plus a
    32-row block swap done by SBUF->SBUF DMA.  Scores are invariant to a
    shared d-permutation of Q and K.
  - Attention is chunk-major (512 queries at a time) per head so scores
    (PE) / exp (Act) / P@V (PE) of many (head, chunk, block) units stay
    in flight together.  Scores stream only the causal range; P@V uses
    exact diagonal sub-ranges.  A ones-augmented V makes row 64 of the
    P@V accumulation the softmax denominator.  No max-subtraction
    (scores are O(6); fp32 exp is safe).
  - V is computed group-major like Q/K (full-width streams, weight loads
    hidden) and transposed to t-major on the PE (cheap 128-col passes).
  - The output projection for a chunk runs right after the last head's
    normalize, so the y DMA streams through phase 2 instead of forming
    a tail.

All matmul inputs fp16 (1 cyc/row on PE); accumulation fp32 in PSUM.
"""

import numpy as np

B, T, C, H = 2, 2048, 1024, 16
HD = C // H            # 64
N_CORES = 8
CPB = 4                # cores per batch
HPC = 4                # heads per core (2 pairs)
TC = 512               # t-chunk for phase 1
NTC = T // TC          # 4
KB = 128               # k-block
NKB = T // KB          # 16
QC = 512               # q-chunk for attention/projection
NQC = T // QC          # 4

_CACHE = {}


def _build_bass():
    import concourse.bacc as bacc
    import concourse.mybir as mybir
    import concourse.tile as tile
    from concourse.masks import make_upper_triangular

    f16 = mybir.dt.float16
    f32 = mybir.dt.float32
    Exp = mybir.ActivationFunctionType.Exp
    Copy = mybir.ActivationFunctionType.Copy
    mult = mybir.AluOpType.mult

    nc = bacc.Bacc()

    xT = nc.dram_tensor("xT", [C, T], f16, kind="ExternalInput")
    wqkv = nc.dram_tensor("wqkv", [C, 768], f16, kind="ExternalInput")
    wp = nc.dram_tensor("wp", [2 * 128, C], f16, kind="ExternalInput")
    trig1 = nc.dram_tensor("trig1", [128, T], f16, kind="ExternalInput")
    trig2 = nc.dram_tensor("trig2", [128, T], f16, kind="ExternalInput")
    y = nc.dram_tensor("y", [T, C], f16, kind="ExternalOutput")

    CCH = C // 128  # 8 contraction chunks

    with tile.TileContext(nc) as tc:
        with (
            tc.tile_pool(name="const", bufs=1) as const,
            tc.tile_pool(name="persist", bufs=1) as persist,
            tc.tile_pool(name="stream", bufs=2) as stream,
            tc.tile_pool(name="ptp", bufs=8) as ptp,
            tc.tile_pool(name="psum", bufs=1, space="PSUM") as psum,
        ):
            # ---- constants; x chunk 0 interleaved with wqkv so the first
            # matmul starts early ----
            wqkv_sb = const.tile([128, CCH, 768], f16)
            wqkv_r = wqkv.rearrange("(cc p) j -> p cc j", p=128)
            x_sb0 = stream.tile([128, CCH, TC], f16, tag="x", name="x_sb")
            xT_r = xT.rearrange("(cc p) t -> p cc t", p=128)
            for cc in range(CCH):
                nc.sync.dma_start(out=wqkv_sb[:, cc, :], in_=wqkv_r[:, cc, :])
                nc.sync.dma_start(out=x_sb0[:, cc, :], in_=xT_r[:, cc, 0:TC])
            trig1_sb = const.tile([128, T], f16)
            nc.scalar.dma_start(out=trig1_sb, in_=trig1[:, :])
            trig2_sb = const.tile([128, T], f16)
            nc.scalar.dma_start(out=trig2_sb, in_=trig2[:, :])
            wp_sb = const.tile([128, 2, C], f16)
            wp_r = wp.rearrange("(p2 p) c -> p p2 c", p=128)
            nc.sync.dma_start(out=wp_sb, in_=wp_r)
            # mask[k, q] = 1 where q >= k (keep), 0 where q < k
            mask_ut = const.tile([128, 128], f16)
            make_upper_triangular(nc, mask_ut, val=1.0, diag=True)

            # ---- persistent tensors ----
            QrotT = persist.tile([128, 2, T], f16)
            KrotT = persist.tile([128, 2, T], f16)
            # V in t-major per (pair, k-block): [V_a(64) | ones | V_b(64) | ones]
            Vaug = persist.tile([128, 2, NKB, 130], f16)
            ones_cols = Vaug.rearrange(
                "pp q J (h x) -> pp q J h x", x=65)[:, :, :, :, 64]
            nc.gpsimd.memset(ones_cols, 1.0)
            Yn = persist.tile([128, 2, T], f16)

            # ================= phase 1: qkv + rotary ======================
            for i in range(NTC):
                ts = slice(i * TC, (i + 1) * TC)
                if i == 0:
                    x_sb = x_sb0
                else:
                    x_sb = stream.tile([128, CCH, TC], f16, tag="x",
                                       name="x_sb")
                    for cc in range(CCH):
                        nc.sync.dma_start(out=x_sb[:, cc, :],
                                          in_=xT_r[:, cc, ts])

                for g in range(4):      # Qp0 Qp1 Kp0 Kp1
                    dst = QrotT if g < 2 else KrotT
                    p = g % 2
                    acc = psum.tile([128, TC], f32, tag="acc", bufs=2,
                                    name="acc")
                    for cc in range(CCH):
                        nc.tensor.matmul(
                            acc, wqkv_sb[:, cc, g * 128:(g + 1) * 128],
                            x_sb[:, cc, :],
                            start=(cc == 0), stop=(cc == CCH - 1))
                    g16 = stream.tile([128, TC], f16, tag="g16")
                    nc.scalar.activation(g16, acc, Copy)
                    # 32-row block swap (evens <-> odds per head) via DMA
                    gsw = stream.tile([128, TC], f16, tag="gsw")
                    for blk in range(4):
                        src = blk ^ 1
                        nc.sync.dma_start(
                            out=gsw[blk * 32:(blk + 1) * 32, :],
                            in_=g16[src * 32:(src + 1) * 32, :])
                    m1 = stream.tile([128, TC], f16, tag="m1")
                    nc.vector.tensor_mul(m1, g16, trig1_sb[:, ts])
                    m2 = stream.tile([128, TC], f16, tag="m2")
                    nc.vector.tensor_mul(m2, gsw, trig2_sb[:, ts])
                    nc.vector.tensor_add(dst[:, p, ts], m1, m2)

                # V computed directly in t-major: x block stationary
                for tb in range(TC // 128):
                    J = i * 4 + tb
                    vacc = psum.tile([128, TC], f32, tag="acc", bufs=2,
                                     name="vacc")
                    for cc in range(CCH):
                        nc.tensor.matmul(
                            vacc[:, 0:256],
                            x_sb[:, cc, tb * 128:(tb + 1) * 128],
                            wqkv_sb[:, cc, 512:768],
                            start=(cc == 0), stop=(cc == CCH - 1))
                    for p in range(2):
                        vdst = Vaug.rearrange(
                            "pp q J (h x) -> pp q J h x",
                            x=65)[:, p, J, :, 0:64]
                        vsrc = vacc[:, p * 128:(p + 1) * 128].rearrange(
                            "pp (h x) -> pp h x", h=2)
                        nc.scalar.activation(vdst, vsrc, Copy)

            # ================= phase 2: attention, chunk-major ============
            for c in range(NQC):
                cs = slice(c * QC, (c + 1) * QC)
                for u in range(HPC):
                    p, hh = divmod(u, 2)
                    hs = slice(hh * 64, hh * 64 + 64)

                    # scores + exp (+ diag mask) for the chunk's k-blocks
                    pts = []
                    for j in range(4 * c + 4):
                        prefix = max(0, (j - 4 * c) * KB)
                        st = psum.tile([128, QC], f32, tag="st", bufs=2,
                                       name="st")
                        nc.tensor.matmul(
                            st[:, prefix:],
                            KrotT[hs, p, j * KB:(j + 1) * KB],
                            QrotT[hs, p, c * QC + prefix:(c + 1) * QC],
                            start=True, stop=True)
                        pt = ptp.tile([128, QC], f16, tag="pt", name="pt")
                        if prefix:
                            nc.gpsimd.memset(pt[:, 0:prefix], 0.0)
                        nc.scalar.activation(pt[:, prefix:], st[:, prefix:],
                                             Exp)
                        if j >= 4 * c:
                            nc.vector.tensor_mul(
                                pt[:, prefix:prefix + 128],
                                pt[:, prefix:prefix + 128], mask_ut)
                        pts.append(pt)

                    # P@V: start=True on j=0 (full width), stop=True on a
                    # full-width piece, partial diagonal pieces in between
                    if c == 0:
                        order = [(j, j == 0, j == 3, 0) for j in range(4)]
                    else:
                        order = [(j, j == 0, False, 0) for j in range(4 * c)]
                        order += [(j, False, False, (j - 4 * c) * KB)
                                  for j in range(4 * c + 1, 4 * c + 4)]
                        order += [(4 * c, False, True, 0)]
                    yps = psum.tile([128, QC], f32, tag="yps", bufs=2,
                                    name="yps")
                    for j, sa, so, pvlo in order:
                        nc.tensor.matmul(
                            yps[0:65, pvlo:],
                            Vaug[:, p, j, hh * 65:(hh + 1) * 65],
                            pts[j][:, pvlo:],
                            start=sa, stop=so)

                    # normalize rows 0-63 by the ones-row (64)
                    dsb = stream.tile([128, QC], f32, tag="dsb")
                    nc.vector.tensor_copy(dsb[0:1, :], yps[64:65, :])
                    rcp = stream.tile([128, QC], f32, tag="rcp")
                    nc.vector.reciprocal_approx_fast(out=rcp[0:1, :],
                                                     in_=dsb[0:1, :])
                    bc = stream.tile([128, QC], f32, tag="bc")
                    nc.gpsimd.partition_broadcast(bc[0:64, :], rcp[0:1, :])
                    nc.vector.tensor_tensor(
                        out=Yn[hs, p, cs], in0=yps[0:64, :], in1=bc[0:64, :],
                        op=mult)

                # ---- projection + output DMA for this chunk ----
                for tt in range(4 * c, 4 * c + 4):
                    tsl = slice(tt * 128, (tt + 1) * 128)
                    for half in range(2):
                        hsl = slice(half * 512, (half + 1) * 512)
                        pout = psum.tile([128, 512], f32, tag="pout",
                                         bufs=2, name="pout")
                        nc.tensor.matmul(pout, Yn[:, 0, tsl],
                                         wp_sb[:, 0, hsl],
                                         start=True, stop=False)
                        nc.tensor.matmul(pout, Yn[:, 1, tsl],
                                         wp_sb[:, 1, hsl],
                                         start=False, stop=True)
                        yo = stream.tile([128, 512], f16, tag="yo", bufs=4)
                        if half == 0:
                            nc.vector.tensor_copy(yo, pout)
                        else:
                            nc.scalar.activation(yo, pout, Copy)
                        nc.sync.dma_start(out=y[tsl, hsl], in_=yo)

    nc.finalize()
    return nc


def _host_prep(x, cos, sin, w_attn, b_attn, w_proj):
    """Per-core input maps (all fp16)."""
    x = np.asarray(x, dtype=np.float32)
    xT16 = [np.ascontiguousarray(x[b].T).astype(np.float16) for b in range(B)]

    cos = np.asarray(cos, dtype=np.float32)  # [T, 32]
    sin = np.asarray(sin, dtype=np.float32)
    cosF = cos.T.astype(np.float16)          # [32, T]
    sinF = sin.T.astype(np.float16)
    trig1 = np.concatenate([cosF, cosF, cosF, cosF], axis=0)   # [128, T]
    trig2 = np.concatenate([-sinF, sinF, -sinF, sinF], axis=0)

    w_attn = np.asarray(w_attn, dtype=np.float32)
    w_proj = np.asarray(w_proj, dtype=np.float32)
    scale = 1.0 / np.sqrt(HD)

    # per-head column permutation: [even dims, odd dims]
    perm = np.concatenate([np.arange(0, HD, 2), np.arange(1, HD, 2)])

    in_maps = []
    for m in range(N_CORES):
        hb = (m % CPB) * HPC
        cols = []
        for g in range(2):           # Q, K: permuted dims, Q scaled
            for pp in range(2):
                for hh in range(2):
                    hglob = hb + pp * 2 + hh
                    blk = w_attn[:, g * C + hglob * HD:
                                 g * C + (hglob + 1) * HD][:, perm]
                    if g == 0:
                        blk = blk * scale
                    cols.append(blk)
        for hh in range(HPC):        # V: natural dims
            hglob = hb + hh
            cols.append(w_attn[:, 2 * C + hglob * HD:
                               2 * C + (hglob + 1) * HD])
        w_stack = np.concatenate(cols, axis=1).astype(np.float16)
        wp_m = w_proj[hb * HD:(hb + HPC) * HD, :].astype(np.float16)
        in_maps.append({"xT": xT16[m // CPB], "wqkv": w_stack, "wp": wp_m,
                        "trig1": trig1, "trig2": trig2})
    return in_maps


def kernel(x, cos, sin, w_attn, b_attn, w_proj, b_proj):
    from concourse.bass_utils import run_bass_kernel_spmd

    b_attn = np.asarray(b_attn, dtype=np.float32)
    assert not np.any(b_attn), "nonzero b_attn not supported by this kernel"

    in_maps = _host_prep(x, cos, sin, w_attn, b_attn, w_proj)

    if "nc" not in _CACHE:
        _CACHE["nc"] = _build_bass()
    nc = _CACHE["nc"]

    res = run_bass_kernel_spmd(nc, in_maps, core_ids=list(range(N_CORES)))
    _CACHE["last_result"] = res

    y = np.zeros((B, T, C), dtype=np.float64)
    for m in range(N_CORES):
        y[m // CPB] += res.results[m]["y"].astype(np.float64)
    y += np.asarray(b_proj, dtype=np.float64)[None, None, :]
    return y.astype(np.float32)


# revision 10
# speedup vs baseline: 1.8598x; 1.1604x over previous
"""Causal self-attention with rotary embeddings on 8 Trainium2 NeuronCores.

Hybrid batch+head tensor parallel: core m handles batch m//4 and heads
[4*(m%4), 4*(m%4)+4).  Each core reads only its batch's x (4 MB), computes
qkv for its 4 heads, rotary, causal attention, and a partial output
projection with its 256 rows of w_proj; the host sums the 4 partial
outputs per batch.

Per-core device layout (heads grouped in 2 pairs):
  - Q^T/K^T per pair: [128 rows = head_a(64) | head_b(64), t].  Within a
    head the 64 dims are permuted to [evens(32), odds(32)] (host permutes
    the w_attn columns), making rotary 3 fp16 DVE tensor ops plus a
    32-row block swap done by SBUF->SBUF DMA.  Scores are invariant to a
    shared d-permutation of Q and K.
  - Attention is chunk-major (512 queries at a time) per head so scores
    (PE) / exp (Act) / P@V (PE) of many (head, chunk, block) units stay
    in flight together.  Scores stream only the causal range; P@V uses
    exact diagonal sub-ranges.  A ones-augmented V makes row 64 of the
    P@V accumulation the softmax denominator.  No max-subtraction
    (scores are O(6); fp32 exp is safe).
  - V is computed group-major like Q/K (full-width streams, weight loads
    hidden) and transposed to t-major on the PE (cheap 128-col passes).
  - The output projection for a chunk runs right after the last head's
    normalize, so the y DMA streams through phase 2 instead of forming
    a tail.

All matmul inputs fp16 (1 cyc/row on PE); accumulation fp32 in PSUM.
"""

import numpy as np

B, T, C, H = 2, 2048, 1024, 16
HD = C // H            # 64
N_CORES = 8
CPB = 4                # cores per batch
HPC = 4                # heads per core (2 pairs)
TC = 512               # t-chunk for phase 1
NTC = T // TC          # 4
KB = 128               # k-block
NKB = T // KB          # 16
QC = 512               # q-chunk for attention/projection
NQC = T // QC          # 4

_CACHE = {}


def _build_bass():
    import concourse.bacc as bacc
    import concourse.mybir as mybir
    import concourse.tile as tile
    from concourse.masks import make_upper_triangular

    f16 = mybir.dt.float16
    f32 = mybir.dt.float32
    Exp = mybir.ActivationFunctionType.Exp
    Copy = mybir.ActivationFunctionType.Copy
    mult = mybir.AluOpType.mult

    nc = bacc.Bacc()

    xT = nc.dram_tensor("xT", [C, T], f16, kind="ExternalInput")
    wqkv = nc.dram_tensor("wqkv", [C, 768], f16, kind="ExternalInput")
    wp = nc.dram_tensor("wp", [2 * 128, C], f16, kind="ExternalInput")
    trig1 = nc.dram_tensor("trig1", [128, T], f16, kind="ExternalInput")
    trig2 = nc.dram_tensor("trig2", [128, T], f16, kind="ExternalInput")
    y = nc.dram_tensor("y", [T, C], f16, kind="ExternalOutput")

    CCH = C // 128  # 8 contraction chunks

    with tile.TileContext(nc) as tc:
        with (
            tc.tile_pool(name="const", bufs=1) as const,
            tc.tile_pool(name="persist", bufs=1) as persist,
            tc.tile_pool(name="stream", bufs=2) as stream,
            tc.tile_pool(name="ptp", bufs=20) as ptp,
            tc.tile_pool(name="psum", bufs=1, space="PSUM") as psum,
        ):
            # ---- constants; x chunk 0 interleaved with wqkv so the first
            # matmul starts early ----
            wqkv_sb = const.tile([128, CCH, 768], f16)
            wqkv_r = wqkv.rearrange("(cc p) j -> p cc j", p=128)
            x_sb0 = stream.tile([128, CCH, TC], f16, tag="x", name="x_sb")
            xT_r = xT.rearrange("(cc p) t -> p cc t", p=128)
            for cc in range(CCH):
                nc.sync.dma_start(out=wqkv_sb[:, cc, :], in_=wqkv_r[:, cc, :])
                nc.sync.dma_start(out=x_sb0[:, cc, :], in_=xT_r[:, cc, 0:TC])
            trig1_sb = const.tile([128, T], f16)
            nc.scalar.dma_start(out=trig1_sb, in_=trig1[:, :])
            trig2_sb = const.tile([128, T], f16)
            nc.scalar.dma_start(out=trig2_sb, in_=trig2[:, :])
            wp_sb = const.tile([128, 2, C], f16)
            wp_r = wp.rearrange("(p2 p) c -> p p2 c", p=128)
            nc.sync.dma_start(out=wp_sb, in_=wp_r)
            # mask[k, q] = 1 where q >= k (keep), 0 where q < k
            mask_ut = const.tile([128, 128], f16)
            make_upper_triangular(nc, mask_ut, val=1.0, diag=True)

            # ---- persistent tensors ----
            QrotT = persist.tile([128, 2, T], f16)
            KrotT = persist.tile([128, 2, T], f16)
            # V in t-major per (pair, k-block): [V_a(64) | ones | V_b(64) | ones]
            Vaug = persist.tile([128, 2, NKB, 130], f16)
            ones_cols = Vaug.rearrange(
                "pp q J (h x) -> pp q J h x", x=65)[:, :, :, :, 64]
            nc.gpsimd.memset(ones_cols, 1.0)
            Yn = persist.tile([128, 2, T], f16)

            # ================= phase 1: qkv + rotary ======================
            for i in range(NTC):
                ts = slice(i * TC, (i + 1) * TC)
                if i == 0:
                    x_sb = x_sb0
                else:
                    x_sb = stream.tile([128, CCH, TC], f16, tag="x",
                                       name="x_sb")
                    for cc in range(CCH):
                        nc.sync.dma_start(out=x_sb[:, cc, :],
                                          in_=xT_r[:, cc, ts])

                for g in range(4):      # Qp0 Qp1 Kp0 Kp1
                    dst = QrotT if g < 2 else KrotT
                    p = g % 2
                    acc = psum.tile([128, TC], f32, tag="acc", bufs=2,
                                    name="acc")
                    for cc in range(CCH):
                        nc.tensor.matmul(
                            acc, wqkv_sb[:, cc, g * 128:(g + 1) * 128],
                            x_sb[:, cc, :],
                            start=(cc == 0), stop=(cc == CCH - 1))
                    g16 = stream.tile([128, TC], f16, tag="g16")
                    nc.scalar.activation(g16, acc, Copy)
                    # 32-row block swap (evens <-> odds per head) via DMA
                    gsw = stream.tile([128, TC], f16, tag="gsw")
                    for blk in range(4):
                        src = blk ^ 1
                        nc.sync.dma_start(
                            out=gsw[blk * 32:(blk + 1) * 32, :],
                            in_=g16[src * 32:(src + 1) * 32, :])
                    m1 = stream.tile([128, TC], f16, tag="m1")
                    nc.vector.tensor_mul(m1, g16, trig1_sb[:, ts])
                    m2 = stream.tile([128, TC], f16, tag="m2")
                    nc.vector.tensor_mul(m2, gsw, trig2_sb[:, ts])
                    nc.vector.tensor_add(dst[:, p, ts], m1, m2)

                # V computed directly in t-major: x block stationary
                for tb in range(TC // 128):
                    J = i * 4 + tb
                    vacc = psum.tile([128, TC], f32, tag="acc", bufs=2,
                                     name="vacc")
                    for cc in range(CCH):
                        nc.tensor.matmul(
                            vacc[:, 0:256],
                            x_sb[:, cc, tb * 128:(tb + 1) * 128],
                            wqkv_sb[:, cc, 512:768],
                            start=(cc == 0), stop=(cc == CCH - 1))
                    for p in range(2):
                        vdst = Vaug.rearrange(
                            "pp q J (h x) -> pp q J h x",
                            x=65)[:, p, J, :, 0:64]
                        vsrc = vacc[:, p * 128:(p + 1) * 128].rearrange(
                            "pp (h x) -> pp h x", h=2)
                        nc.scalar.activation(vdst, vsrc, Copy)

            # ================= phase 2: attention, chunk-major ============
            for c in range(NQC):
                cs = slice(c * QC, (c + 1) * QC)
                for u in range(HPC):
                    p, hh = divmod(u, 2)
                    hs = slice(hh * 64, hh * 64 + 64)

                    # scores + exp (+ diag mask) for the chunk's k-blocks
                    pts = []
                    for j in range(4 * c + 4):
                        prefix = max(0, (j - 4 * c) * KB)
                        st = psum.tile([128, QC], f32, tag="st", bufs=2,
                                       name="st")
                        nc.tensor.matmul(
                            st[:, prefix:],
                            KrotT[hs, p, j * KB:(j + 1) * KB],
                            QrotT[hs, p, c * QC + prefix:(c + 1) * QC],
                            start=True, stop=True)
                        pt = ptp.tile([128, QC], f16, tag="pt", name="pt")
                        if prefix:
                            nc.gpsimd.memset(pt[:, 0:prefix], 0.0)
                        nc.scalar.activation(pt[:, prefix:], st[:, prefix:],
                                             Exp)
                        if j >= 4 * c:
                            nc.vector.tensor_mul(
                                pt[:, prefix:prefix + 128],
                                pt[:, prefix:prefix + 128], mask_ut)
                        pts.append(pt)

                    # P@V: start=True on j=0 (full width), stop=True on a
                    # full-width piece, partial diagonal pieces in between
                    if c == 0:
                        order = [(j, j == 0, j == 3, 0) for j in range(4)]
                    else:
                        order = [(j, j == 0, False, 0) for j in range(4 * c)]
                        order += [(j, False, False, (j - 4 * c) * KB)
                                  for j in range(4 * c + 1, 4 * c + 4)]
                        order += [(4 * c, False, True, 0)]
                    yps = psum.tile([128, QC], f32, tag="yps", bufs=2,
                                    name="yps")
                    for j, sa, so, pvlo in order:
                        nc.tensor.matmul(
                            yps[0:65, pvlo:],
                            Vaug[:, p, j, hh * 65:(hh + 1) * 65],
                            pts[j][:, pvlo:],
                            start=sa, stop=so)

                    # normalize rows 0-63 by the ones-row (64)
                    dsb = stream.tile([128, QC], f32, tag="dsb")
                    nc.vector.tensor_copy(dsb[0:1, :], yps[64:65, :])
                    rcp = stream.tile([128, QC], f32, tag="rcp")
                    nc.vector.reciprocal_approx_fast(out=rcp[0:1, :],
                                                     in_=dsb[0:1, :])
                    bc = stream.tile([128, QC], f32, tag="bc")
                    nc.gpsimd.partition_broadcast(bc[0:64, :], rcp[0:1, :])
                    nc.vector.tensor_tensor(
                        out=Yn[hs, p, cs], in0=yps[0:64, :], in1=bc[0:64, :],
                        op=mult)

                # ---- projection + output DMA for this chunk ----
                for tt in range(4 * c, 4 * c + 4):
                    tsl = slice(tt * 128, (tt + 1) * 128)
                    for half in range(2):
                        hsl = slice(half * 512, (half + 1) * 512)
                        pout = psum.tile([128, 512], f32, tag="pout",
                                         bufs=2, name="pout")
                        nc.tensor.matmul(pout, Yn[:, 0, tsl],
                                         wp_sb[:, 0, hsl],
                                         start=True, stop=False)
                        nc.tensor.matmul(pout, Yn[:, 1, tsl],
                                         wp_sb[:, 1, hsl],
                                         start=False, stop=True)
                        yo = stream.tile([128, 512], f16, tag="yo", bufs=4)
                        if half == 0:
                            nc.vector.tensor_copy(yo, pout)
                        else:
                            nc.scalar.activation(yo, pout, Copy)
                        nc.sync.dma_start(out=y[tsl, hsl], in_=yo)

    nc.finalize()
    return nc


def _host_prep(x, cos, sin, w_attn, b_attn, w_proj):
    """Per-core input maps (all fp16)."""
    x = np.asarray(x, dtype=np.float32)
    xT16 = [np.ascontiguousarray(x[b].T).astype(np.float16) for b in range(B)]

    cos = np.asarray(cos, dtype=np.float32)  # [T, 32]
    sin = np.asarray(sin, dtype=np.float32)
    cosF = cos.T.astype(np.float16)          # [32, T]
    sinF = sin.T.astype(np.float16)
    trig1 = np.concatenate([cosF, cosF, cosF, cosF], axis=0)   # [128, T]
    trig2 = np.concatenate([-sinF, sinF, -sinF, sinF], axis=0)

    w_attn = np.asarray(w_attn, dtype=np.float32)
    w_proj = np.asarray(w_proj, dtype=np.float32)
    scale = 1.0 / np.sqrt(HD)

    # per-head column permutation: [even dims, odd dims]
    perm = np.concatenate([np.arange(0, HD, 2), np.arange(1, HD, 2)])

    in_maps = []
    for m in range(N_CORES):
        hb = (m % CPB) * HPC
        cols = []
        for g in range(2):           # Q, K: permuted dims, Q scaled
            for pp in range(2):
                for hh in range(2):
                    hglob = hb + pp * 2 + hh
                    blk = w_attn[:, g * C + hglob * HD:
                                 g * C + (hglob + 1) * HD][:, perm]
                    if g == 0:
                        blk = blk * scale
                    cols.append(blk)
        for hh in range(HPC):        # V: natural dims
            hglob = hb + hh
            cols.append(w_attn[:, 2 * C + hglob * HD:
                               2 * C + (hglob + 1) * HD])
        w_stack = np.concatenate(cols, axis=1).astype(np.float16)
        wp_m = w_proj[hb * HD:(hb + HPC) * HD, :].astype(np.float16)
        in_maps.append({"xT": xT16[m // CPB], "wqkv": w_stack, "wp": wp_m,
                        "trig1": trig1, "trig2": trig2})
    return in_maps


def kernel(x, cos, sin, w_attn, b_attn, w_proj, b_proj):
    from concourse.bass_utils import run_bass_kernel_spmd

    b_attn = np.asarray(b_attn, dtype=np.float32)
    assert not np.any(b_attn), "nonzero b_attn not supported by this kernel"

    in_maps = _host_prep(x, cos, sin, w_attn, b_attn, w_proj)

    if "nc" not in _CACHE:
        _CACHE["nc"] = _build_bass()
    nc = _CACHE["nc"]

    res = run_bass_kernel_spmd(nc, in_maps, core_ids=list(range(N_CORES)))
    _CACHE["last_result"] = res

    y = np.zeros((B, T, C), dtype=np.float64)
    for m in range(N_CORES):
        y[m // CPB] += res.results[m]["y"].astype(np.float64)
    y += np.asarray(b_proj, dtype=np.float64)[None, None, :]
    return y.astype(np.float32)
